# revision 3
# baseline (speedup 1.0000x reference)
"""Trainium2 Bass kernel for nn_ActorNetSpiking (4-layer spiking actor net).

Strategy
--------
Data-parallel over batch: 8 NeuronCores x 512 rows each. Everything on-chip
lives in [feature, batch] layout so each layer's spike output is directly the
next layer's matmul moving operand (contraction dim on partitions, no
transposes anywhere).

Numerics: each weight matrix W is represented as fp16(W) + fp16(W - fp16(W)).
fp16 x fp16 products are exact on the PE (bit-exact incl. subnormals, verified
on HW) and accumulate in fp32 PSUM, so a 2-pass matmul reproduces fp32-level
matmul accuracy at full bf16-rate (1 cycle/row). Layer-1 input x is split the
same way (3 passes: hi*xhi + hi*xres + res*xhi).

Spiking recurrence is rewritten in shifted form to eliminate per-step bias
adds and the (1-s) complement:
    b_eff = b + W.sum(1)     (the W.sum term folds W @ s = W@1 - W@r)
    u' := u - 2 b_eff  ->  u'_t = 0.5 u'_{t-1} - (W @ r_t)     [+W1@x for L1]
    w  := v - 2 b_eff  ->  w_t = 0.75 vr_{t-1} + u'_t
    r_t = (w_t <= 0.5 - 2 b_eff)          (complement spike, fp16, feeds matmul)
    vr_t = (w_t + 2 b_eff) * r_t          (reset membrane, the v-carry)
    acc_t = (acc_{t-1} + 1) - r4_t
Each neuron update is 3 DVE ops + 1 GpSimd op per [128, 512] tile.

Weights for layers 2/3 are streamed from DRAM every step (they don't fit in
SBUF next to the 96KB/partition of fp32 state); x streams per step. All
streamed DRAM tensors are laid out partition-major so each partition's data is
one contiguous descriptor.
"""

import sys

sys.path.insert(0, "/opt/trn_rl_repo")

import numpy as np

# ---- problem constants (hardcoded per contract) ----
B, S, T = 4096, 512, 50
H = 1024
A = 2
NCORES = 8
BS = B // NCORES          # 512 batch rows per core
P = 128                   # partitions
KT1 = S // P              # 4 k-tiles for layer 1
KT = H // P               # 8 k-tiles for layers 2-4
HT = H // P               # 8 h-tiles for layers 1-3
NB = BS                   # matmul free dim

CDECAY, VDECAY, VTH = 0.5, 0.75, 0.5
F16_MIN_NORMAL = 6.104e-5

_CACHE = {}


def _f16pair(a):
    """a (fp32) -> (hi fp16, res fp16) with hi+res ~ a to ~2^-24 abs."""
    hi = a.astype(np.float16).astype(np.float32)
    hi[np.abs(a) < 2 * F16_MIN_NORMAL] = 0.0
    res = (a - hi).astype(np.float16)
    return hi.astype(np.float16), res


def _build_program():
    import concourse.mybir as mybir
    import concourse.tile as tile
    from concourse import bacc

    f32 = mybir.dt.float32
    f16 = mybir.dt.float16
    AOT = mybir.AluOpType

    nc = bacc.Bacc("TRN2", target_bir_lowering=False, debug=False)

    # ---- DRAM tensors (all streamed tensors partition-major contiguous) ----
    xd = nc.dram_tensor("x", (T, P, 2 * KT1 * NB), f16, kind="ExternalInput")
    w1d = nc.dram_tensor("w1", (P, 2 * KT1 * H), f16, kind="ExternalInput")
    w2d = nc.dram_tensor("w2", (HT, P, 2 * KT * P), f16, kind="ExternalInput")
    w3d = nc.dram_tensor("w3", (HT, P, 2 * KT * P), f16, kind="ExternalInput")
    w4d = nc.dram_tensor("w4", (P, 2 * KT * A), f16, kind="ExternalInput")
    u0d = nc.dram_tensor("u0", (3, P, HT * NB), f32, kind="ExternalInput")
    thrd = nc.dram_tensor("thr", (P, 3 * HT), f32, kind="ExternalInput")
    twobd = nc.dram_tensor("twob", (P, 3 * HT), f32, kind="ExternalInput")
    l4cd = nc.dram_tensor("l4c", (A, 3), f32, kind="ExternalInput")  # thr4|twob4|u40
    outd = nc.dram_tensor("out", (A, BS), f32, kind="ExternalOutput")

    with tile.TileContext(nc) as tc:
        with (
            tc.tile_pool(name="const", bufs=1) as cp,
            tc.tile_pool(name="state", bufs=1) as stp,
            tc.tile_pool(name="xp", bufs=2) as xp,
            tc.tile_pool(name="wcol", bufs=6) as wcp,
            tc.tile_pool(name="rp", bufs=3) as rp,
            tc.tile_pool(name="wv", bufs=4) as wvp,
            tc.tile_pool(name="l4t", bufs=2) as l4p,
            tc.tile_pool(name="ps", bufs=7, space="PSUM") as pp,
            tc.tile_pool(name="ps4", bufs=1, space="PSUM") as pp4,
        ):
            # ---- resident constants ----
            w1sb = cp.tile([P, 2, KT1, H], f16)
            nc.sync.dma_start(
                w1sb[:], w1d.ap().rearrange("p (c k h) -> p c k h", c=2, k=KT1)
            )
            w4sb = cp.tile([P, 2, KT, A], f16)
            nc.sync.dma_start(
                w4sb[:], w4d.ap().rearrange("p (c k a) -> p c k a", c=2, k=KT)
            )
            thrsb = cp.tile([P, 3 * HT], f32)
            nc.sync.dma_start(thrsb[:], thrd.ap())
            twobsb = cp.tile([P, 3 * HT], f32)
            nc.sync.dma_start(twobsb[:], twobd.ap())
            l4c = cp.tile([A, 3], f32)
            nc.sync.dma_start(l4c[:], l4cd.ap())

            # ---- states ----
            u_st = [stp.tile([P, HT * NB], f32, tag=f"u{l}", name=f"u{l}") for l in range(3)]
            vr_st = [stp.tile([P, HT * NB], f32, tag=f"vr{l}", name=f"vr{l}") for l in range(3)]
            for l in range(3):
                nc.sync.dma_start(u_st[l][:], u0d.ap()[l])
                nc.vector.memset(vr_st[l][:], 0.0)
            u4 = stp.tile([A, NB], f32, tag="u4")
            vr4 = stp.tile([A, NB], f32, tag="vr4")
            acc = stp.tile([A, NB], f32, tag="acc")
            # u4 init = -2*be4 broadcast along free dim
            nc.vector.memset(u4[:], 0.0)
            nc.vector.tensor_scalar(u4[:], u4[:], l4c[:, 2:3], None, op0=AOT.add)
            nc.vector.memset(vr4[:], 0.0)
            nc.vector.memset(acc[:], 0.0)

            def neuron(l, j, ps, r_tile):
                """Shifted-state neuron update for layer l (0-2), h-tile j."""
                sl = slice(j * NB, (j + 1) * NB)
                u_sl = u_st[l][:, sl]
                vr_sl = vr_st[l][:, sl]
                cj = l * HT + j
                nc.vector.scalar_tensor_tensor(
                    u_sl, u_sl, CDECAY, ps[:], op0=AOT.mult, op1=AOT.add
                )
                wv = wvp.tile([P, NB], f32, tag="wv")
                nc.vector.scalar_tensor_tensor(
                    wv[:], vr_sl, VDECAY, u_sl, op0=AOT.mult, op1=AOT.add
                )
                nc.vector.tensor_scalar(
                    r_tile[:, j, :], wv[:], thrsb[:, cj : cj + 1], None, op0=AOT.is_le
                )
                nc.vector.scalar_tensor_tensor(
                    vr_sl, wv[:], twobsb[:, cj : cj + 1], r_tile[:, j, :],
                    op0=AOT.add, op1=AOT.mult,
                )

            mm = nc.tensor.matmul
            for t in range(T):
                xt = xp.tile([P, 2, KT1, NB], f16, tag="xt")
                nc.sync.dma_start(
                    xt[:], xd.ap()[t].rearrange("p (c k b) -> p c k b", c=2, k=KT1)
                )
                # ---- layer 1 ----
                r1 = rp.tile([P, KT, NB], f16, tag="r")
                for j in range(HT):
                    hs = slice(j * P, (j + 1) * P)
                    ps = pp.tile([P, NB], f32, tag="ps")
                    for k in range(KT1):
                        mm(ps[:], w1sb[:, 0, k, hs], xt[:, 0, k, :],
                           start=(k == 0), stop=False)
                        mm(ps[:], w1sb[:, 0, k, hs], xt[:, 1, k, :],
                           start=False, stop=False)
                    for k in range(KT1):
                        mm(ps[:], w1sb[:, 1, k, hs], xt[:, 0, k, :],
                           start=False, stop=(k == KT1 - 1))
                    neuron(0, j, ps, r1)
                # ---- layers 2, 3 ----
                r_prev = r1
                for li, wdram in ((1, w2d), (2, w3d)):
                    r_new = rp.tile([P, KT, NB], f16, tag="r")
                    for j in range(HT):
                        wc = wcp.tile([P, 2, KT, P], f16, tag="wc")
                        eng = nc.sync if (j % 2 == 0) else nc.scalar
                        eng.dma_start(
                            wc[:],
                            wdram.ap()[j].rearrange("p (c k q) -> p c k q", c=2, k=KT),
                        )
                        ps = pp.tile([P, NB], f32, tag="ps")
                        for k in range(KT):
                            mm(ps[:], wc[:, 0, k, :], r_prev[:, k, :],
                               start=(k == 0), stop=False)
                            mm(ps[:], wc[:, 1, k, :], r_prev[:, k, :],
                               start=False, stop=(k == KT - 1))
                        neuron(li, j, ps, r_new)
                    r_prev = r_new
                # ---- layer 4 ----
                ps4 = pp4.tile([A, NB], f32, tag="ps4")
                for k in range(KT):
                    mm(ps4[:], w4sb[:, 0, k, :], r_prev[:, k, :],
                       start=(k == 0), stop=False)
                    mm(ps4[:], w4sb[:, 1, k, :], r_prev[:, k, :],
                       start=False, stop=(k == KT - 1))
                nc.vector.scalar_tensor_tensor(
                    u4[:], u4[:], CDECAY, ps4[:], op0=AOT.mult, op1=AOT.add
                )
                wv4 = l4p.tile([A, NB], f32, tag="wv4")
                nc.vector.scalar_tensor_tensor(
                    wv4[:], vr4[:], VDECAY, u4[:], op0=AOT.mult, op1=AOT.add
                )
                r4 = l4p.tile([A, NB], f32, tag="r4")
                nc.vector.tensor_scalar(
                    r4[:], wv4[:], l4c[:, 0:1], None, op0=AOT.is_le
                )
                nc.vector.scalar_tensor_tensor(
                    vr4[:], wv4[:], l4c[:, 1:2], r4[:], op0=AOT.add, op1=AOT.mult
                )
                nc.vector.scalar_tensor_tensor(
                    acc[:], acc[:], 1.0, r4[:], op0=AOT.add, op1=AOT.subtract
                )

            nc.sync.dma_start(outd.ap(), acc[:])

    nc.compile()
    return nc


def _prep_shared(W1, b1, W2, b2, W3, b3, W4, b4):
    """Host-side weight/constant prep shared by all cores."""
    def beff(W, b, fold):
        c = W.astype(np.float64).sum(axis=1)
        return (b.astype(np.float64) + (c if fold else 0.0))

    be = [
        beff(W1, b1, False),
        beff(W2, b2, True),
        beff(W3, b3, True),
        beff(W4, b4, True),
    ]

    def tile_w(WT_f32, kt):
        """WT [K, Hout] fp32 -> partition-major (Hout//P, P, 2, kt, P) fp16."""
        hi, res = _f16pair(WT_f32)
        K, Ho = WT_f32.shape
        out = np.empty((Ho // P, P, 2, kt, P), np.float16)
        for c, arr in ((0, hi), (1, res)):
            # arr [K, Ho] -> [kt, P(k), Ho//P, P(h)] -> [hcol, P(k), kt, P(h)]
            a4 = arr.reshape(kt, P, Ho // P, P)
            out[:, :, c, :, :] = np.transpose(a4, (2, 1, 0, 3))
        return np.ascontiguousarray(out.reshape(Ho // P, P, 2 * kt * P))

    w1hi, w1res = _f16pair(np.ascontiguousarray(W1.T))  # [S, H]
    w1t = np.empty((P, 2, KT1, H), np.float16)
    w1t[:, 0] = np.transpose(w1hi.reshape(KT1, P, H), (1, 0, 2))
    w1t[:, 1] = np.transpose(w1res.reshape(KT1, P, H), (1, 0, 2))
    w1t = np.ascontiguousarray(w1t.reshape(P, 2 * KT1 * H))

    w2t = tile_w(np.ascontiguousarray((-W2).T), KT)
    w3t = tile_w(np.ascontiguousarray((-W3).T), KT)

    w4hi, w4res = _f16pair(np.ascontiguousarray((-W4).T))  # [K, A]
    w4t = np.empty((P, 2, KT, A), np.float16)
    w4t[:, 0] = np.transpose(w4hi.reshape(KT, P, A), (1, 0, 2))
    w4t[:, 1] = np.transpose(w4res.reshape(KT, P, A), (1, 0, 2))
    w4t = np.ascontiguousarray(w4t.reshape(P, 2 * KT * A))

    # thresholds / shifts, layout [P, l*HT+j] with feature h = j*P + p
    thr = np.empty((P, 3 * HT), np.float32)
    twob = np.empty((P, 3 * HT), np.float32)
    u0 = np.empty((3, P, HT * NB), np.float32)
    for l in range(3):
        be_l = be[l]
        for j in range(HT):
            fv = be_l[j * P : (j + 1) * P]
            thr[:, l * HT + j] = (VTH - 2.0 * fv).astype(np.float32)
            twob[:, l * HT + j] = (2.0 * fv).astype(np.float32)
            u0[l, :, j * NB : (j + 1) * NB] = np.float32(-2.0) * np.broadcast_to(
                fv.astype(np.float32)[:, None], (P, NB)
            )
    l4c = np.stack(
        [
            (VTH - 2.0 * be[3]).astype(np.float32),
            (2.0 * be[3]).astype(np.float32),
            (-2.0 * be[3]).astype(np.float32),
        ],
        axis=1,
    )  # [A, 3]
    return dict(w1=w1t, w2=w2t, w3=w3t, w4=w4t, thr=thr,
                twob=np.ascontiguousarray(twob), u0=np.ascontiguousarray(u0),
                l4c=np.ascontiguousarray(l4c))


def _prep_x_core(xc):
    """xc [BS, S, T] fp32 -> [T, P, 2*KT1*NB] fp16 (hi|res, partition-major)."""
    xt = np.transpose(xc, (2, 1, 0)).astype(np.float32)  # [T, S, BS]
    hi = xt.astype(np.float16)
    res = (xt - hi.astype(np.float32)).astype(np.float16)
    out = np.empty((T, P, 2, KT1, NB), np.float16)
    for c, arr in ((0, hi), (1, res)):
        out[:, :, c, :, :] = np.transpose(arr.reshape(T, KT1, P, NB), (0, 2, 1, 3))
    return np.ascontiguousarray(out.reshape(T, P, 2 * KT1 * NB))


def _get_nc():
    if "nc" not in _CACHE:
        _CACHE["nc"] = _build_program()
    return _CACHE["nc"]


def kernel(x, W1, b1, W2, b2, W3, b3, W4, b4, batch_size, _trace=False):
    from concourse.bass_utils import run_bass_kernel_spmd

    x = np.asarray(x, np.float32)
    W1, b1 = np.asarray(W1, np.float32), np.asarray(b1, np.float32)
    W2, b2 = np.asarray(W2, np.float32), np.asarray(b2, np.float32)
    W3, b3 = np.asarray(W3, np.float32), np.asarray(b3, np.float32)
    W4, b4 = np.asarray(W4, np.float32), np.asarray(b4, np.float32)
    assert x.shape == (B, S, T)

    nc = _get_nc()
    shared = _prep_shared(W1, b1, W2, b2, W3, b3, W4, b4)
    in_maps = []
    for c in range(NCORES):
        m = dict(shared)
        m["x"] = _prep_x_core(x[c * BS : (c + 1) * BS])
        in_maps.append(m)

    res = run_bass_kernel_spmd(
        nc, in_maps, core_ids=list(range(NCORES)), trace=_trace
    )
    _CACHE["last_results"] = res
    out = np.empty((B, A), np.float32)
    for c in range(NCORES):
        out[c * BS : (c + 1) * BS] = res.results[c]["out"].T
    return out / np.float32(T)


# revision 7
# speedup vs baseline: 24.6551x; 24.6551x over previous
"""Trainium2 Bass kernel for nn_ActorNetSpiking (4-layer spiking actor net).

Strategy
--------
Data-parallel over batch: 8 NeuronCores x 512 rows each. Everything on-chip
lives in [feature, batch] layout so each layer's spike output is directly the
next layer's matmul moving operand (contraction dim on partitions, no
transposes anywhere).

Numerics: each weight matrix W is represented as fp16(W) + fp16(W - fp16(W)).
fp16 x fp16 products are exact on the PE (bit-exact incl. subnormals, verified
on HW) and accumulate in fp32 PSUM, so a 2-pass matmul reproduces fp32-level
matmul accuracy at full rate (~140-210 ns per [128x128]@[128x512]). Layer-1
input x is split the same way (3 passes: hi*xhi + hi*xres + res*xhi).

Spiking recurrence in shifted form (eliminates per-step bias adds and the
(1-s) complement):
    b_eff = b + W.sum(1)     (folds W @ s = W@1 - W@r into constants)
    u' := u - 2 b_eff  ->  u'_t = 0.5 u'_{t-1} + psum_t
    w  := v - 2 b_eff  ->  w_t = 0.75 vr_{t-1} + u'_t
    r_t = (w_t <= 0.5 - 2 b_eff)        (complement spike, fp16, feeds matmul)
    vr_t = (w_t + 2 b_eff) * r_t        (true reset membrane, the v-carry)
    acc_t = (acc_{t-1} + 1) - r4_t
Neuron update: 4 DVE ops per [128, 512] tile (3x scalar_tensor_tensor +
1x tensor_scalar compare).

SBUF (per partition, ~208KB budget): fp32 u/vr states for layers 1-3 = 96KB,
W1 hi+res and W2/W3 hi parts resident = 48KB; W2/W3 fp16 residual parts are
streamed from DRAM each step (2MB/layer/step), x streamed per step. All
streamed DRAM tensors are partition-major so each partition's data is one
contiguous DMA descriptor.
"""

import sys

sys.path.insert(0, "/opt/trn_rl_repo")

import numpy as np

# ---- problem constants (hardcoded per contract) ----
B, S, T = 4096, 512, 50
H = 1024
A = 2
NCORES = 8
BS = B // NCORES          # 512 batch rows per core
P = 128                   # partitions
KT1 = S // P              # 4 k-tiles for layer 1
KT = H // P               # 8 k-tiles for layers 2-4
HT = H // P               # 8 h-tiles for layers 1-3
NB = BS                   # matmul free dim

CDECAY, VDECAY, VTH = 0.5, 0.75, 0.5
F16_MIN_NORMAL = 6.104e-5

REPEAT = 1             # timing experiments only: repeat the scan in one NEFF
DIAG_SKELETON = False  # timing diagnostics: op1-only neuron, constant r

_CACHE = {}


def _f16pair(a):
    """a (fp32) -> (hi fp16, res fp16) with hi+res ~ a to ~2^-24 abs."""
    hi = a.astype(np.float16).astype(np.float32)
    hi[np.abs(a) < 2 * F16_MIN_NORMAL] = 0.0
    res = (a - hi).astype(np.float16)
    return hi.astype(np.float16), res


def _build_program():
    import concourse.mybir as mybir
    import concourse.tile as tile
    from concourse import bacc

    f32 = mybir.dt.float32
    f16 = mybir.dt.float16
    AOT = mybir.AluOpType

    nc = bacc.Bacc("TRN2", target_bir_lowering=False, debug=False)

    # ---- DRAM tensors (streamed tensors are partition-major contiguous) ----
    xd = nc.dram_tensor("x", (T, P, 2 * KT1 * NB), f16, kind="ExternalInput")
    w1d = nc.dram_tensor("w1", (P, 2 * KT1 * H), f16, kind="ExternalInput")
    w2hid = nc.dram_tensor("w2hi", (P, KT * H), f16, kind="ExternalInput")
    w3hid = nc.dram_tensor("w3hi", (P, KT * H), f16, kind="ExternalInput")
    w2resd = nc.dram_tensor("w2res", (HT, P, KT * P), f16, kind="ExternalInput")
    w3resd = nc.dram_tensor("w3res", (HT, P, KT * P), f16, kind="ExternalInput")
    w4d = nc.dram_tensor("w4", (P, 2 * KT * A), f16, kind="ExternalInput")
    u0d = nc.dram_tensor("u0", (3, P, HT * NB), f32, kind="ExternalInput")
    thrd = nc.dram_tensor("thr", (P, 3 * HT), f32, kind="ExternalInput")
    twobd = nc.dram_tensor("twob", (P, 3 * HT), f32, kind="ExternalInput")
    l4cd = nc.dram_tensor("l4c", (A, 3), f32, kind="ExternalInput")  # thr|twob|u0
    outd = nc.dram_tensor("out", (A, BS), f32, kind="ExternalOutput")

    with tile.TileContext(nc) as tc:
        with (
            tc.tile_pool(name="const", bufs=1) as cp,
            tc.tile_pool(name="state", bufs=1) as stp,
            tc.tile_pool(name="xp", bufs=2) as xp,
            tc.tile_pool(name="wcol", bufs=6) as wcp,
            tc.tile_pool(name="rp", bufs=2) as rp,
            tc.tile_pool(name="wv", bufs=3) as wvp,
            tc.tile_pool(name="l4t", bufs=1) as l4p,
            tc.tile_pool(name="ps", bufs=7, space="PSUM") as pp,
            tc.tile_pool(name="ps4", bufs=1, space="PSUM") as pp4,
        ):
            # ---- resident weights / constants ----
            w1sb = cp.tile([P, 2, KT1, H], f16)
            nc.sync.dma_start(
                w1sb[:], w1d.ap().rearrange("p (c k h) -> p c k h", c=2, k=KT1)
            )
            w2hisb = cp.tile([P, KT, H], f16)
            nc.sync.dma_start(
                w2hisb[:], w2hid.ap().rearrange("p (k h) -> p k h", k=KT)
            )
            w3hisb = cp.tile([P, KT, H], f16)
            nc.sync.dma_start(
                w3hisb[:], w3hid.ap().rearrange("p (k h) -> p k h", k=KT)
            )
            w4sb = cp.tile([P, 2, KT, A], f16)
            nc.sync.dma_start(
                w4sb[:], w4d.ap().rearrange("p (c k a) -> p c k a", c=2, k=KT)
            )
            thrsb = cp.tile([P, 3 * HT], f32)
            nc.sync.dma_start(thrsb[:], thrd.ap())
            twobsb = cp.tile([P, 3 * HT], f32)
            nc.sync.dma_start(twobsb[:], twobd.ap())
            l4c = cp.tile([A, 3], f32)
            nc.sync.dma_start(l4c[:], l4cd.ap())

            # ---- states ----
            u_st = [stp.tile([P, HT * NB], f32, tag=f"u{l}", name=f"u{l}")
                    for l in range(3)]
            vr_st = [stp.tile([P, HT * NB], f32, tag=f"vr{l}", name=f"vr{l}")
                     for l in range(3)]
            for l in range(3):
                nc.sync.dma_start(u_st[l][:], u0d.ap()[l])
                nc.vector.memset(vr_st[l][:], 0.0)
            u4 = stp.tile([A, NB], f32, tag="u4")
            vr4 = stp.tile([A, NB], f32, tag="vr4")
            acc = stp.tile([A, NB], f32, tag="acc")
            nc.vector.memset(u4[:], 0.0)
            nc.vector.tensor_scalar(u4[:], u4[:], l4c[:, 2:3], None, op0=AOT.add)
            nc.vector.memset(vr4[:], 0.0)
            nc.vector.memset(acc[:], 0.0)

            rconst = None
            if DIAG_SKELETON:
                rconst = cp.tile([P, KT, NB], f16, name="rconst")
                nc.vector.memset(rconst[:], 1.0)

            def neuron(l, j, ps, r_tile):
                """Shifted-state neuron update for layer l (0-2), h-tile j."""
                sl = slice(j * NB, (j + 1) * NB)
                u_sl = u_st[l][:, sl]
                vr_sl = vr_st[l][:, sl]
                cj = l * HT + j
                nc.vector.scalar_tensor_tensor(
                    u_sl, u_sl, CDECAY, ps[:], op0=AOT.mult, op1=AOT.add
                )
                if DIAG_SKELETON:
                    return
                wv = wvp.tile([P, NB], f32, tag="wv")
                nc.vector.scalar_tensor_tensor(
                    wv[:], vr_sl, VDECAY, u_sl, op0=AOT.mult, op1=AOT.add
                )
                nc.vector.tensor_scalar(
                    r_tile[:, j, :], wv[:], thrsb[:, cj : cj + 1], None,
                    op0=AOT.is_le,
                )
                nc.vector.scalar_tensor_tensor(
                    vr_sl, wv[:], twobsb[:, cj : cj + 1], r_tile[:, j, :],
                    op0=AOT.add, op1=AOT.mult,
                )

            mm = nc.tensor.matmul
            for t in [tt for _ in range(REPEAT) for tt in range(T)]:
                xt = xp.tile([P, 2, KT1, NB], f16, tag="xt")
                nc.sync.dma_start(
                    xt[:], xd.ap()[t].rearrange("p (c k b) -> p c k b", c=2, k=KT1)
                )
                # ---- layer 1 (hi*xhi + hi*xres + res*xhi) ----
                r1 = rconst if DIAG_SKELETON else rp.tile([P, KT, NB], f16, tag="r")
                for j in range(HT):
                    hs = slice(j * P, (j + 1) * P)
                    ps = pp.tile([P, NB], f32, tag="ps")
                    for k in range(KT1):
                        mm(ps[:], w1sb[:, 0, k, hs], xt[:, 0, k, :],
                           start=(k == 0), stop=False)
                        mm(ps[:], w1sb[:, 0, k, hs], xt[:, 1, k, :],
                           start=False, stop=False)
                    for k in range(KT1):
                        mm(ps[:], w1sb[:, 1, k, hs], xt[:, 0, k, :],
                           start=False, stop=(k == KT1 - 1))
                    neuron(0, j, ps, r1)
                # ---- layers 2, 3 (hi resident, res streamed per h-column) ----
                r_prev = r1
                for li, whisb, wresd in ((1, w2hisb, w2resd), (2, w3hisb, w3resd)):
                    r_new = (rconst if DIAG_SKELETON
                             else rp.tile([P, KT, NB], f16, tag="r"))
                    for j in range(HT):
                        hs = slice(j * P, (j + 1) * P)
                        wc = wcp.tile([P, KT, P], f16, tag="wc")
                        eng = nc.sync if (j % 2 == 0) else nc.scalar
                        eng.dma_start(
                            wc[:],
                            wresd.ap()[j].rearrange("p (k q) -> p k q", k=KT),
                        )
                        ps = pp.tile([P, NB], f32, tag="ps")
                        for k in range(KT):
                            mm(ps[:], whisb[:, k, hs], r_prev[:, k, :],
                               start=(k == 0), stop=False)
                            mm(ps[:], wc[:, k, :], r_prev[:, k, :],
                               start=False, stop=(k == KT - 1))
                        neuron(li, j, ps, r_new)
                    r_prev = r_new
                # ---- layer 4 ----
                ps4 = pp4.tile([A, NB], f32, tag="ps4")
                for k in range(KT):
                    mm(ps4[:], w4sb[:, 0, k, :], r_prev[:, k, :],
                       start=(k == 0), stop=False)
                    mm(ps4[:], w4sb[:, 1, k, :], r_prev[:, k, :],
                       start=False, stop=(k == KT - 1))
                nc.vector.scalar_tensor_tensor(
                    u4[:], u4[:], CDECAY, ps4[:], op0=AOT.mult, op1=AOT.add
                )
                if DIAG_SKELETON:
                    continue
                wv4 = l4p.tile([A, NB], f32, tag="wv4")
                nc.vector.scalar_tensor_tensor(
                    wv4[:], vr4[:], VDECAY, u4[:], op0=AOT.mult, op1=AOT.add
                )
                r4 = l4p.tile([A, NB], f32, tag="r4")
                nc.vector.tensor_scalar(
                    r4[:], wv4[:], l4c[:, 0:1], None, op0=AOT.is_le
                )
                nc.vector.scalar_tensor_tensor(
                    vr4[:], wv4[:], l4c[:, 1:2], r4[:], op0=AOT.add, op1=AOT.mult
                )
                nc.vector.scalar_tensor_tensor(
                    acc[:], acc[:], 1.0, r4[:], op0=AOT.add, op1=AOT.subtract
                )

            nc.sync.dma_start(outd.ap(), acc[:])

    nc.compile()
    return nc


def _prep_shared(W1, b1, W2, b2, W3, b3, W4, b4):
    """Host-side weight/constant prep shared by all cores."""
    def beff(W, b, fold):
        c = W.astype(np.float64).sum(axis=1)
        return b.astype(np.float64) + (c if fold else 0.0)

    be = [
        beff(W1, b1, False),
        beff(W2, b2, True),
        beff(W3, b3, True),
        beff(W4, b4, True),
    ]

    w1hi, w1res = _f16pair(np.ascontiguousarray(W1.T))  # [S, H]
    w1t = np.empty((P, 2, KT1, H), np.float16)
    w1t[:, 0] = np.transpose(w1hi.reshape(KT1, P, H), (1, 0, 2))
    w1t[:, 1] = np.transpose(w1res.reshape(KT1, P, H), (1, 0, 2))
    w1t = np.ascontiguousarray(w1t.reshape(P, 2 * KT1 * H))

    def hi_res(W):
        WT = np.ascontiguousarray((-W).T)  # [K, Ho]
        hi, res = _f16pair(WT)
        K, Ho = WT.shape
        hit = np.ascontiguousarray(
            np.transpose(hi.reshape(KT, P, Ho), (1, 0, 2)).reshape(P, KT * Ho)
        )
        # res per h-column j: [HT, P, KT*P], rest[j, p, k*P+q] = res[k*P+p, j*P+q]
        r4d = res.reshape(KT, P, Ho // P, P)
        rest = np.ascontiguousarray(
            np.transpose(r4d, (2, 1, 0, 3)).reshape(Ho // P, P, KT * P)
        )
        return hit, rest

    w2hit, w2rest = hi_res(W2)
    w3hit, w3rest = hi_res(W3)

    w4hi, w4res = _f16pair(np.ascontiguousarray((-W4).T))  # [K, A]
    w4t = np.empty((P, 2, KT, A), np.float16)
    w4t[:, 0] = np.transpose(w4hi.reshape(KT, P, A), (1, 0, 2))
    w4t[:, 1] = np.transpose(w4res.reshape(KT, P, A), (1, 0, 2))
    w4t = np.ascontiguousarray(w4t.reshape(P, 2 * KT * A))

    # shifted-form constants, layout [P, l*HT+j] with feature h = j*P + p
    thr = np.empty((P, 3 * HT), np.float32)
    twob = np.empty((P, 3 * HT), np.float32)
    u0 = np.empty((3, P, HT * NB), np.float32)
    for l in range(3):
        for j in range(HT):
            fv = be[l][j * P : (j + 1) * P]
            thr[:, l * HT + j] = (VTH - 2.0 * fv).astype(np.float32)
            twob[:, l * HT + j] = (2.0 * fv).astype(np.float32)
            u0[l, :, j * NB : (j + 1) * NB] = np.broadcast_to(
                (-2.0 * fv).astype(np.float32)[:, None], (P, NB)
            )
    l4c = np.stack(
        [
            (VTH - 2.0 * be[3]).astype(np.float32),
            (2.0 * be[3]).astype(np.float32),
            (-2.0 * be[3]).astype(np.float32),
        ],
        axis=1,
    )  # [A, 3]
    return dict(w1=w1t, w2hi=w2hit, w2res=w2rest, w3hi=w3hit, w3res=w3rest,
                w4=w4t, thr=thr, twob=np.ascontiguousarray(twob),
                u0=np.ascontiguousarray(u0), l4c=np.ascontiguousarray(l4c))


def _prep_x_core(xc):
    """xc [BS, S, T'] fp32 -> [T', P, 2*KT1*NB] fp16 (hi|res, partition-major)."""
    Tc = xc.shape[2]
    xt = np.transpose(xc, (2, 1, 0)).astype(np.float32)  # [T', S, BS]
    hi = xt.astype(np.float16)
    res = (xt - hi.astype(np.float32)).astype(np.float16)
    out = np.empty((Tc, P, 2, KT1, NB), np.float16)
    for c, arr in ((0, hi), (1, res)):
        out[:, :, c, :, :] = np.transpose(arr.reshape(Tc, KT1, P, NB), (0, 2, 1, 3))
    return np.ascontiguousarray(out.reshape(Tc, P, 2 * KT1 * NB))


def _get_nc():
    if "nc" not in _CACHE:
        _CACHE["nc"] = _build_program()
    return _CACHE["nc"]


def kernel(x, W1, b1, W2, b2, W3, b3, W4, b4, batch_size, _trace=False):
    from concourse.bass_utils import run_bass_kernel_spmd

    x = np.asarray(x, np.float32)
    W1, b1 = np.asarray(W1, np.float32), np.asarray(b1, np.float32)
    W2, b2 = np.asarray(W2, np.float32), np.asarray(b2, np.float32)
    W3, b3 = np.asarray(W3, np.float32), np.asarray(b3, np.float32)
    W4, b4 = np.asarray(W4, np.float32), np.asarray(b4, np.float32)
    assert x.shape == (B, S, T)

    nc = _get_nc()
    shared = _prep_shared(W1, b1, W2, b2, W3, b3, W4, b4)
    in_maps = []
    for c in range(NCORES):
        m = dict(shared)
        m["x"] = _prep_x_core(x[c * BS : (c + 1) * BS])
        in_maps.append(m)

    res = run_bass_kernel_spmd(
        nc, in_maps, core_ids=list(range(NCORES)), trace=_trace
    )
    _CACHE["last_results"] = res
    out = np.empty((B, A), np.float32)
    for c in range(NCORES):
        out[c * BS : (c + 1) * BS] = res.results[c]["out"].T
    return out / np.float32(T)
